# revision 10
# baseline (speedup 1.0000x reference)
"""ClusterISAAttention Trainium2 kernel (8 NeuronCores, SPMD) — v2.

Host: per batch (2), stable-sort queries by window id, split into 4
contiguous quarters of 2048 queries -> 8 (batch, quarter) shards, one
per core.  Queries grouped into slots: one window per slot, <=32
queries; slot count padded to fixed NSLOT=96 (measured max over the 8
shards).  Slot keys zero-padded to KP=64 in xpP so 2 slots = 128 key
partitions.  V-projection bias is folded into the output bias on the
host (softmax rows sum to 1): bo' = Wo @ bv + bo.

Device (per core), bf16 matmuls with fp32 psum, per phase (12 slots):
  kT    = Wk-halves @ xpP(49 valid cols/slot) -> dense [128d, 12, 49]
          (+bk via one activation), then 8 SBUF->SBUF DMAs scatter the
          4 head-blocks into block-diag kbd[g][128, 12, 4, 49]
  V     = keys-major: stationary xpP slot-pair [128c,128k] x wvT
          -> psum [128k, 256d] -> V_sb bf16; 4 strided DMAs build
          block-diag vbd[128(2a x 64k), 4x, 12, 64(2a x 32d)]
  S     = qT-slot.T @ kbd -> psum [128(4sl x 32q), 2, 4, 49]
          (tile_position row packing, 49-packed columns)
  A     = exp(S) -> a_sb[128q, 4x, 3t, 128(2a x 64k)], key pads stay 0
          (memset once per pool slot); den = rowsum(0:49), A *= 1/den
  atbd  = sync-xbar transpose of A -> [128k, 4x, 3t, 4cls, 32q]
  ctx   = atbd-stationary masking trick: stationary [128k, 128q] per
          (tile,x) streams vbd 4 slots -> psum [128q, 4x, 4sl, 64d];
          valid diagonal (query-slot == streamed-slot) extracted by 12
          copies/phase (engines round-robin) -> ctxU [128q, 3t, 256d]
  ctxT  = sync-xbar transpose of ctxU -> [128d, 2, q] dims-major
  outT  = Wo @ ctxT + bo' -> fp32 [256, NQ] -> DRAM
Host: outT columns scattered back to original query order.

All xbar transposes go through nc.sync ONLY (concurrent transposes on
two HWDGE rings corrupt each other - HW-verified); plain DMAs ride the
gpsimd SWDGE queues.  xbar map: src elem i -> dst partition i%128, dst
mid i//128.
"""

import os
import sys
import numpy as np
import ml_dtypes

for _p in ("/opt/trn_rl_repo", "/root/.axon_site/_ro/trn_rl_repo"):
    if os.path.isdir(_p) and _p not in sys.path:
        sys.path.append(_p)

import concourse.bass as bass
import concourse.tile as tile
from concourse import bacc
from concourse import mybir

F32 = mybir.dt.float32
BF16 = mybir.dt.bfloat16
AF = mybir.ActivationFunctionType
ALU = mybir.AluOpType
AX = mybir.AxisListType

B, N, C, H, HD, W, K = 2, 8192, 256, 8, 32, 361, 49
SCALE = float(HD) ** -0.5

NCORES = 8
QTRS = 4
NLOC = N // QTRS         # 2048 queries per core
CAP = 32                 # queries per slot
NSLOT = 96               # fixed slot count (measured max = 96)
NQ = NSLOT * CAP         # 3072 padded queries
PH = 8                   # phases (double-buffered)
SPP = NSLOT // PH        # 12 slots per phase
TPP = SPP // 4           # 3 four-slot tiles per phase
KP = 64                  # padded keys per slot (v-side partition align)

DEBUG_DUMP = False


def _build_program():
    nc = bacc.Bacc("TRN2", target_bir_lowering=False, debug=False,
                   num_devices=NCORES)

    xqT = nc.declare_dram_parameter("xqT", [C, NQ], BF16, isOutput=False).ap()
    xpP = nc.declare_dram_parameter("xpP", [C, NSLOT * KP], BF16, isOutput=False).ap()
    wts = {
        nm: nc.declare_dram_parameter(f"w{nm}T", [C, C], BF16, isOutput=False).ap()
        for nm in ("q", "k", "v", "o")
    }
    bss = {
        nm: nc.declare_dram_parameter(f"b{nm}", [C], F32, isOutput=False).ap()
        for nm in ("q", "k", "o")
    }
    outT = nc.declare_dram_parameter("outT", [C, NQ], F32, isOutput=True).ap()
    dbg = None
    if DEBUG_DUMP:
        dbg = {
            "d_qT": nc.declare_dram_parameter("d_qT", [2, 128, NQ], F32, isOutput=True).ap(),
            "d_kbd": nc.declare_dram_parameter("d_kbd", [2, 128, SPP * 4 * K], F32, isOutput=True).ap(),
            "d_vbd": nc.declare_dram_parameter("d_vbd", [4, 128, SPP * KP], F32, isOutput=True).ap(),
            "d_a": nc.declare_dram_parameter("d_a", [128, 4 * TPP * 2 * KP], F32, isOutput=True).ap(),
            "d_atbd": nc.declare_dram_parameter("d_atbd", [128, 4 * TPP * 4 * CAP], F32, isOutput=True).ap(),
            "d_ctxU": nc.declare_dram_parameter("d_ctxU", [128, TPP * 256], F32, isOutput=True).ap(),
            "d_ctxT": nc.declare_dram_parameter("d_ctxT", [128, 2 * NQ], F32, isOutput=True).ap(),
        }

    with tile.TileContext(nc) as tc:
        _kernel_body(tc, xqT, xpP, wts, bss, outT, dbg)
    nc.compile()
    return nc


def _kernel_body(tc, xqT, xpP, wts, bss, outT, dbg=None):
    from contextlib import ExitStack

    nc = tc.nc
    ctx = ExitStack()
    with ctx:
        singles = ctx.enter_context(tc.tile_pool(name="singles", bufs=1))
        phpool = ctx.enter_context(tc.tile_pool(name="phase", bufs=2))
        pp = ctx.enter_context(tc.tile_pool(name="proj_ps", bufs=2, space="PSUM"))
        sp = ctx.enter_context(tc.tile_pool(name="s_ps", bufs=2, space="PSUM"))
        cp = ctx.enter_context(tc.tile_pool(name="ctx_ps", bufs=2, space="PSUM"))
        ostage = ctx.enter_context(tc.tile_pool(name="ostage", bufs=2))

        # ---- persistent SBUF ----
        w_sb = {nm: singles.tile([128, 2, C], BF16, tag=f"w_{nm}", name=f"w_{nm}") for nm in wts}
        b_sb = {nm: singles.tile([128, 2], F32, tag=f"b_{nm}", name=f"b_{nm}") for nm in bss}
        xq_sb = [singles.tile([128, NQ], BF16, tag=f"xq{c}", name=f"xq{c}") for c in range(2)]
        xp_sb = [singles.tile([128, NSLOT * KP], BF16, tag=f"xp{c}", name=f"xp{c}") for c in range(2)]
        qT_sb = [singles.tile([128, NQ], BF16, tag=f"qT{g}", name=f"qT{g}") for g in range(2)]
        ctxT_sb = singles.tile([128, 2, NQ], BF16, tag="ctxT", name="ctxT")

        # ---- load inputs (SWDGE; keep Sync free for xbar transposes) ----
        for nm in wts:
            nc.gpsimd.dma_start(
                out=w_sb[nm][:], in_=wts[nm].rearrange("(s p) m -> p s m", p=128))
        for nm in bss:
            nc.gpsimd.dma_start(
                out=b_sb[nm][:], in_=bss[nm].rearrange("(s p) -> p s", p=128))
        for c in range(2):
            nc.gpsimd.dma_start(out=xq_sb[c][:], in_=xqT[c * 128:(c + 1) * 128, :])
            nc.gpsimd.dma_start(out=xp_sb[c][:], in_=xpP[c * 128:(c + 1) * 128, :])

        # ---- q projection ----
        for nch in range(NQ // 512):
            for m in range(2):
                ps = pp.tile([128, 512], F32, tag="ps", name="ps")
                for c in range(2):
                    nc.tensor.matmul(
                        ps[:], w_sb["q"][:, c, m * 128:(m + 1) * 128],
                        xq_sb[c][:, nch * 512:(nch + 1) * 512],
                        start=(c == 0), stop=(c == 1))
                nc.scalar.activation(
                    qT_sb[m][:, nch * 512:(nch + 1) * 512], ps[:], AF.Identity,
                    bias=b_sb["q"][:, m:m + 1], scale=SCALE)

        last = {}
        for ph in range(PH):
            # ---- phase tiles (pool-cycled, 2 slots per tag) ----
            # kbd[g]: [128 (4b x 32d), slot, b, 49] block-diag: rows 32b
            # pair with b-plane, other planes stay zero (memset once).
            kbd_sb = [phpool.tile([128, SPP, 4, K], BF16, tag=f"kbd{g}", name=f"kbd{g}") for g in range(2)]
            kt_sb = [phpool.tile([128, SPP, K], BF16, tag=f"kt{m}", name=f"kt{m}") for m in range(2)]
            # V_sb: keys-major V for 6 slot-pairs [128 (2sl x 64k), sp, 256d]
            v_sb = phpool.tile([128, SPP // 2, C], BF16, tag="v_sb", name="v_sb")
            # vbd[x]: [128 (2a x 64k), slot, (2a x 32d)] block-diag per slot
            vbd_sb = [phpool.tile([128, SPP, KP], BF16, tag=f"vbd{x}", name=f"vbd{x}") for x in range(4)]
            a_sb = phpool.tile([128, 4, TPP, 2 * KP], BF16, tag="a_sb", name="a_sb")
            atbd_sb = phpool.tile([128, 4, TPP, 4, CAP], BF16, tag="atbd", name="atbd")
            ctxu_sb = phpool.tile([128, TPP, C], BF16, tag="ctxu", name="ctxu")
            den_sb = phpool.tile([128, TPP, 8], F32, tag="den", name="den")
            rec_sb = phpool.tile([128, TPP, 8], F32, tag="rec", name="rec")
            last = dict(kbd_sb=kbd_sb, vbd_sb=vbd_sb, a_sb=a_sb,
                        atbd_sb=atbd_sb, ctxu_sb=ctxu_sb)

            if ph < 2:
                # zero the never-written block-diag gaps once per pool slot;
                # the slot layout repeats every phase so zeros persist.
                for g in range(2):
                    nc.gpsimd.memset(kbd_sb[g][:], 0.0)
                for x in range(4):
                    nc.gpsimd.memset(vbd_sb[x][:], 0.0)
                # key pads 49:64 of a_sb (exp never writes them)
                nc.vector.memset(
                    a_sb[:].rearrange("p x t (a j) -> p x t a j", a=2)[:, :, :, :, K:KP], 0.0)

            # ---- k projection: dense kT, bias folded, then DMA scatter ----
            for m in range(2):
                for half in range(2):  # 6 slots per chunk
                    s0 = half * (SPP // 2)
                    ps = pp.tile([128, 512], F32, tag="ps", name="ps")
                    psv = ps[:, 0:6 * K].rearrange("p (s k) -> p s k", k=K)
                    for c in range(2):
                        src = xp_sb[c][:, (ph * SPP + s0) * KP:(ph * SPP + s0 + 6) * KP] \
                            .rearrange("p (s j) -> p s j", j=KP)[:, :, 0:K]
                        nc.tensor.matmul(
                            psv, w_sb["k"][:, c, m * 128:(m + 1) * 128],
                            src, start=(c == 0), stop=(c == 1))
                    nc.scalar.activation(
                        kt_sb[m][:, s0:s0 + 6, :], psv, AF.Identity,
                        bias=b_sb["k"][:, m:m + 1])
                # scatter 4 head-blocks into kbd via SWDGE (no engine cost)
                for b in range(4):
                    nc.gpsimd.dma_start(
                        out=kbd_sb[m][32 * b:32 * b + 32, :, b, :],
                        in_=kt_sb[m][32 * b:32 * b + 32, :, :])

            # ---- v projection: keys-major via xpP-pair stationaries ----
            for sp_i in range(SPP // 2):
                col0 = (ph * SPP + 2 * sp_i) * KP
                ps = pp.tile([128, 512], F32, tag="ps", name="ps")
                for c in range(2):
                    nc.tensor.matmul(
                        ps[:, 0:C], xp_sb[c][:, col0:col0 + 2 * KP],
                        w_sb["v"][:, c, :], start=(c == 0), stop=(c == 1))
                if sp_i % 2 == 0:
                    nc.scalar.copy(v_sb[:, sp_i, :], ps[:, 0:C])
                else:
                    nc.vector.tensor_copy(v_sb[:, sp_i, :], ps[:, 0:C])
            # vbd build: 16 strided DMAs (a = key-half / head-in-pair, par =
            # slot parity, x = head pair), spread over 4 DGE rings.  head
            # h = 4*(x//2) + 2*(x%2) + a has dims at V_sb col 64x + 32a,
            # keys at V_sb rows 64*(sl%2).
            rings = (nc.gpsimd, nc.scalar, nc.sync, nc.gpsimd)
            for x in range(4):
                for a in range(2):
                    for par in range(2):
                        rings[x].dma_start(
                            out=vbd_sb[x][64 * a:64 * a + 64, par::2, 32 * a:32 * a + 32],
                            in_=v_sb[64 * par:64 * par + 64, :, 64 * x + 32 * a:64 * x + 32 * a + 32])

            # ---- S = q.k per 4-slot tile (49-packed psum) ----
            for t in range(TPP):
                st = sp.tile([128, 512], F32, tag="st", name="st")
                stv = st[:, 0:2 * 4 * K].rearrange("p (g b k) -> p g b k", g=2, b=4)
                for sl in range(4):
                    s_ph = t * 4 + sl
                    qcol = (ph * SPP + s_ph) * CAP
                    for g in range(2):
                        nc.tensor.matmul(
                            stv[32 * sl:32 * sl + 32, g, :, :],
                            qT_sb[g][:, qcol:qcol + CAP],
                            kbd_sb[g][:, s_ph, :, :],
                            start=(g == 0), stop=(g == 1),
                            skip_group_check=True, tile_position=(0, 32 * sl))
                # exp into 64-strided a_sb (pads stay zero)
                nc.scalar.activation(
                    a_sb[:, :, t, :].rearrange("p (gb) (a j) -> p gb a j", a=2)[:, :, :, 0:K],
                    stv.rearrange("p g b k -> p (g b) k")
                       .rearrange("p (gb a) k -> p gb a k", a=2),
                    AF.Exp)
                nc.vector.tensor_reduce(
                    out=den_sb[:, t, :].rearrange("p (x a) -> p x a", x=4),
                    in_=a_sb[:, :, t, :].rearrange("p x (a j) -> p x a j", a=2)[:, :, :, 0:K],
                    axis=AX.X, op=ALU.add)
            nc.vector.reciprocal(
                rec_sb[:].rearrange("p a b -> p (a b)"),
                den_sb[:].rearrange("p a b -> p (a b)"))
            for t in range(TPP):
                a4 = a_sb[:, :, t, :].rearrange("p x (a j) -> p x a j", a=2)
                r4 = rec_sb[:, t, :].rearrange("p (x a) -> p x a", x=4) \
                    .unsqueeze(3).broadcast_to([128, 4, 2, KP])
                nc.vector.tensor_tensor(out=a4, in0=a4, in1=r4, op=ALU.mult)

            # ---- A -> atbd: one xbar transpose per query-class ----
            for cq in range(4):
                nc.sync.dma_start_transpose(
                    out=atbd_sb[:, :, :, cq, :],
                    in_=a_sb[32 * cq:32 * cq + 32, :, :, :])

            # ---- ctx: atbd-stationary masking trick, 4 MMs per tile ----
            for t in range(TPP):
                cps = cp.tile([128, 4, 4, KP], F32, tag="cps", name="cps")
                for x in range(4):
                    nc.tensor.matmul(
                        cps[:, x, :, :],
                        atbd_sb[:, x, t, :, :],
                        vbd_sb[x][:, 4 * t:4 * t + 4, :],
                        start=True, stop=True, skip_group_check=True)
                # diagonal extraction (query-slot == streamed-slot)
                for s in range(4):
                    src = cps[32 * s:32 * s + 32, :, s, :]
                    dst = ctxu_sb[32 * s:32 * s + 32, t, :] \
                        .rearrange("p (x j) -> p x j", x=4)
                    eng = (nc.vector, nc.scalar, nc.vector, nc.scalar)[s]
                    if eng is nc.scalar:
                        eng.copy(dst, src)
                    else:
                        eng.tensor_copy(dst, src)
            # ---- ctxU -> ctxT: one xbar transpose per tile ----
            for t in range(TPP):
                qcol = (ph * TPP + t) * 128
                nc.sync.dma_start_transpose(
                    out=ctxT_sb[:, :, qcol:qcol + 128],
                    in_=ctxu_sb[:, t, :])

        if dbg is not None:
            dpool = ctx.enter_context(tc.tile_pool(name="dbgpool", bufs=2))

            def dump(dst, src_ap):
                nd = len(src_ap.shape)
                if nd == 3:
                    src_ap = src_ap.rearrange("p a b -> p (a b)")
                elif nd == 4:
                    src_ap = src_ap.rearrange("p a b c -> p (a b c)")
                elif nd == 5:
                    src_ap = src_ap.rearrange("p a b c d -> p (a b c d)")
                fs = src_ap.shape[1]
                for c0 in range(0, fs, 512):
                    w = min(512, fs - c0)
                    stg = dpool.tile([128, 512], F32, name="dstg", tag="dstg")
                    nc.vector.tensor_copy(stg[:, 0:w], src_ap[:, c0:c0 + w])
                    nc.gpsimd.dma_start(out=dst[:, c0:c0 + w], in_=stg[:, 0:w])

            for g in range(2):
                dump(dbg["d_qT"][g], qT_sb[g][:])
                dump(dbg["d_kbd"][g], last["kbd_sb"][g][:])
            for x in range(4):
                dump(dbg["d_vbd"][x], last["vbd_sb"][x][:])
            dump(dbg["d_a"], last["a_sb"][:])
            dump(dbg["d_atbd"], last["atbd_sb"][:])
            dump(dbg["d_ctxU"], last["ctxu_sb"][:])
            dump(dbg["d_ctxT"], ctxT_sb[:])

        # ---- output projection ----
        for nch in range(NQ // 512):
            for m in range(2):
                ps = pp.tile([128, 512], F32, tag="ps", name="ps")
                for c in range(2):
                    nc.tensor.matmul(
                        ps[:], w_sb["o"][:, c, m * 128:(m + 1) * 128],
                        ctxT_sb[:, c, nch * 512:(nch + 1) * 512],
                        start=(c == 0), stop=(c == 1))
                ot = ostage.tile([128, 512], F32, tag="ot", name="ot")
                nc.scalar.activation(ot[:], ps[:], AF.Identity,
                                     bias=b_sb["o"][:, m:m + 1])
                nc.gpsimd.dma_start(
                    out=outT[m * 128:(m + 1) * 128, nch * 512:(nch + 1) * 512],
                    in_=ot[:])


_PROGRAM = None


def _get_program():
    global _PROGRAM
    if _PROGRAM is None:
        _PROGRAM = _build_program()
    return _PROGRAM


def _pack_core(x_b, xp_b, qidx, wins):
    slot_win = []
    slot_q = []
    i = 0
    n = len(qidx)
    while i < n:
        w = wins[i]
        j = i
        while j < n and wins[j] == w:
            j += 1
        for s in range(i, j, CAP):
            slot_win.append(w)
            slot_q.append(qidx[s:min(s + CAP, j)])
        i = j
    assert len(slot_win) <= NSLOT, f"slot overflow: {len(slot_win)}"
    nreal = len(slot_win)

    owner = np.full([NQ], -1, np.int64)
    xq = np.zeros([NQ, C], np.float32)
    for si, qs in enumerate(slot_q):
        if len(qs):
            xq[si * CAP: si * CAP + len(qs)] = x_b[qs]
            owner[si * CAP: si * CAP + len(qs)] = qs
    xqT = np.ascontiguousarray(xq.T).astype(ml_dtypes.bfloat16)
    # keys zero-padded 49 -> 64 per slot; dummy slots stay all-zero
    xpp = np.zeros([NSLOT, KP, C], np.float32)
    xpp[:nreal, 0:K, :] = xp_b[np.asarray(slot_win)]
    xpP = np.ascontiguousarray(
        xpp.reshape(NSLOT * KP, C).T).astype(ml_dtypes.bfloat16)
    return xqT, xpP, owner


def make_in_maps(x, x_permute, idx_win, Wq, bq, Wk, bk, Wv, bv, Wo, bo):
    x = np.asarray(x, np.float32)
    xp = np.asarray(x_permute, np.float32)
    idx = np.asarray(idx_win)
    Wo32 = np.asarray(Wo, np.float32)
    bo_f = Wo32 @ np.asarray(bv, np.float32) + np.asarray(bo, np.float32)
    shared = {
        "wqT": np.ascontiguousarray(np.asarray(Wq, np.float32).T).astype(ml_dtypes.bfloat16),
        "wkT": np.ascontiguousarray(np.asarray(Wk, np.float32).T).astype(ml_dtypes.bfloat16),
        "wvT": np.ascontiguousarray(np.asarray(Wv, np.float32).T).astype(ml_dtypes.bfloat16),
        "woT": np.ascontiguousarray(Wo32.T).astype(ml_dtypes.bfloat16),
        "bq": (np.asarray(bq, np.float32) * SCALE).astype(np.float32),
        "bk": np.asarray(bk, np.float32),
        "bo": bo_f.astype(np.float32),
    }
    in_maps, owners = [], []
    for core in range(NCORES):
        b, qtr = divmod(core, QTRS)
        order = np.argsort(idx[b], kind="stable")
        qidx = order[qtr * NLOC:(qtr + 1) * NLOC]
        wins = idx[b][qidx]
        xqT, xpP, owner = _pack_core(x[b], xp[b], qidx, wins)
        in_maps.append({"xqT": xqT, "xpP": xpP, **shared})
        owners.append((b, owner))
    return in_maps, owners


def kernel(x, x_permute, idx_win, Wq, bq, Wk, bk, Wv, bv, Wo, bo):
    from concourse.bass_utils import run_bass_kernel_spmd

    nc = _get_program()
    in_maps, owners = make_in_maps(
        x, x_permute, idx_win, Wq, bq, Wk, bk, Wv, bv, Wo, bo)
    res = run_bass_kernel_spmd(nc, in_maps, list(range(NCORES)))
    out = np.zeros([B, N, C], np.float32)
    for core in range(NCORES):
        b, owner = owners[core]
        oT = np.asarray(res.results[core]["outT"], np.float32)
        valid = owner >= 0
        out[b][owner[valid]] = oT.T[valid]
    return out
